# revision 17
# baseline (speedup 1.0000x reference)
"""GRU (Keras reset_after=True) encoder kernel for 8 trn2 NeuronCores.

Strategy:
  - x@W projection (xg) sharded across cores by time, then one AllGather.
  - The sequential recurrence is REPLICATED on all 8 cores (per-step
    collectives are too expensive: ~4.6us floor x 512 steps).
  - Per step, the recurrent matmul h @ U streams U through the PE with the
    batch (64) as the stationary operand, 2-way column-tiled so PE array
    columns 0-63 serve the H-halve jh=0 and columns 64-127 serve jh=1.
  - xz/xr are accumulated into PSUM via an identity matmul so sigmoid reads
    PSUM directly; gates are pipelined in 4 blocks of 128 columns so PE
    transposes + next-step matmuls start before all gates finish.

Layouts (host-prepared):
  gate tensors on chip: [p = b + 64*jh, j0] with j = jh*512 + j0, H=1024.
  psum rg: [128, 1536] = [z 512 | r 512 | h 512] (cols j0 per half jh(p)).
  hT_buf:  [128, 4, 128]; slot [:, c, jh*64:+64] = h^T for h-chunk 4*jh+c.
"""

import os
import sys

import numpy as np

for _p in ("/opt/trn_rl_repo",):
    if _p not in sys.path and os.path.isdir(_p):
        sys.path.insert(0, _p)

B, T, D, H = 64, 512, 256, 1024
NCORES = 8
H2 = H // 2  # 512

_CACHE = {}


def _build(T_steps, with_bias_pre, with_bias_rh, trace_scopes=False):
    import concourse.bass as bass
    import concourse.tile as tile
    from concourse import bacc, mybir
    from concourse.masks import make_identity

    f32 = mybir.dt.float32
    f16 = mybir.dt.float16
    T_loc = T_steps // NCORES

    nc = bacc.Bacc("TRN2", target_bir_lowering=False, debug=False,
                   num_devices=NCORES)

    # ---- external I/O ----
    xT_e = nc.dram_tensor("xT", [T_loc, 128, 2, 64], f16, kind="ExternalInput").ap()
    Up_e = nc.dram_tensor("Up", [128, 2, 8, 1536], f16, kind="ExternalInput").ap()
    Wp_e = nc.dram_tensor("Wp", [128, 2, 2, 1536], f16, kind="ExternalInput").ap()
    h0bt_e = nc.dram_tensor("h0bt", [128, H2], f32, kind="ExternalInput").ap()
    h0T_e = nc.dram_tensor("h0T", [128, 4, 128], f16, kind="ExternalInput").ap()
    if with_bias_pre:
        bpre_e = nc.dram_tensor("bpre", [128, 1536], f32, kind="ExternalInput").ap()
    if with_bias_rh:
        brh_e = nc.dram_tensor("brh", [128, H2], f32, kind="ExternalInput").ap()
    ys_e = nc.dram_tensor("ys", [T_steps, 128, H2], f32, kind="ExternalOutput").ap()

    with tile.TileContext(nc) as tc, \
            tc.tile_pool(name="dram", bufs=1, space="DRAM") as dramp:
        xg_shard = dramp.tile([T_loc, 128, 1536], f16, name="xg_shard")
        xg_full = dramp.tile([T_steps, 128, 1536], f16, name="xg_full",
                             addr_space="Shared")
        singles_cm = tc.tile_pool(name="singles", bufs=1)
        singles = singles_cm.__enter__()
        # constants (live for the whole kernel)
        ident = singles.tile([128, 128], f16)
        make_identity(nc, ident)
        ident32 = singles.tile([128, 128], f32)
        make_identity(nc, ident32)
        Up_sb = singles.tile([128, 2, 8, 1536], f16)
        for jh in range(2):
            for k in range(8):
                nc.sync.dma_start(Up_sb[:, jh, k], Up_e[:, jh, k])
        h0_sb = singles.tile([128, H2], f32)
        nc.sync.dma_start(h0_sb[:], h0bt_e[:])
        hT0_sb = singles.tile([128, 4, 128], f16)
        nc.sync.dma_start(hT0_sb[:], h0T_e[:])
        if with_bias_pre:
            bpre_sb = singles.tile([128, 1536], f32)
            nc.sync.dma_start(bpre_sb[:], bpre_e[:])
        if with_bias_rh:
            brh_sb = singles.tile([128, H2], f32)
            nc.sync.dma_start(brh_sb[:], brh_e[:])

        with (
            tc.tile_pool(name="presing", bufs=1) as presing,
            tc.tile_pool(name="xk", bufs=3) as xkp,
            tc.tile_pool(name="xgsb", bufs=3) as xgsbp,
            tc.tile_pool(name="prepsum", bufs=2, space="PSUM") as prepsum,
        ):
            Wp_sb = presing.tile([128, 2, 2, 1536], f16)
            for jh in range(2):
                for dk in range(2):
                    nc.sync.dma_start(Wp_sb[:, jh, dk], Wp_e[:, jh, dk])

            # ---- phase 1: xg = x @ W (+ bias folds) for local t-shard ----
            for it in range(T_loc):
                xk = xkp.tile([128, 2, 64], f16, tag="xk")
                nc.sync.dma_start(xk[:], xT_e[it])
                ps = prepsum.tile([128, 1536], f32, tag="ps")
                for dk in range(2):
                    st = dk == 0
                    sp = dk == 1
                    for ns in range(3):
                        nsl = slice(ns * 512, ns * 512 + 512)
                        nc.tensor.matmul(
                            ps[0:64, nsl], xk[:, dk, :], Wp_sb[:, 0, dk, nsl],
                            start=st, stop=sp, tile_position=(0, 0))
                        nc.tensor.matmul(
                            ps[64:128, nsl], xk[:, dk, :], Wp_sb[:, 1, dk, nsl],
                            start=st, stop=sp, tile_position=(0, 64))
                xg_sb = xgsbp.tile([128, 1536], f16, tag="xg_sb")
                for ns in range(3):
                    nsl = slice(ns * 512, ns * 512 + 512)
                    if with_bias_pre:
                        nc.vector.tensor_add(
                            out=xg_sb[:, nsl], in0=ps[:, nsl], in1=bpre_sb[:, nsl])
                    else:
                        nc.vector.tensor_copy(out=xg_sb[:, nsl], in_=ps[:, nsl])
                nc.sync.dma_start(xg_shard[it, :, 0:768], xg_sb[:, 0:768])
                nc.sync.dma_start(xg_shard[it, :, 768:1536], xg_sb[:, 768:1536])

            # ---- phase 2: allgather xg ----
            nc.gpsimd.collective_compute(
                "AllGather",
                mybir.AluOpType.bypass,
                ins=[xg_shard[:].opt()],
                outs=[xg_full[:].opt()],
                replica_groups=[list(range(NCORES))],
            )

        # ---- phase 3: recurrence ----
        with (
            tc.tile_pool(name="xgt", bufs=4) as xgtp,
            tc.tile_pool(name="hpool", bufs=3) as hpool,
            tc.tile_pool(name="hTpool", bufs=2) as hTpool,
            tc.tile_pool(name="gt", bufs=8) as gtp,
            tc.tile_pool(name="rgps", bufs=2, space="PSUM") as rgpsp,
            tc.tile_pool(name="trps", bufs=2, space="PSUM") as trpsp,
        ):
            h_prev = h0_sb
            hT_prev = hT0_sb
            Sigmoid = mybir.ActivationFunctionType.Sigmoid
            Tanh = mybir.ActivationFunctionType.Tanh

            def fetch_xg(t):
                xg_t = xgtp.tile([128, 1536], f16, tag="xg_t")
                for q in range(4):
                    csl = slice(q * 384, q * 384 + 384)
                    nc.sync.dma_start(xg_t[:, csl], xg_full[t, :, csl])
                return xg_t

            def id_mms(ps, xg_t):
                # identity matmuls fold xz/xr into psum (cols 0:1024)
                for ns in range(2):
                    nsl = slice(ns * 512, ns * 512 + 512)
                    nc.tensor.matmul(ps[0:64, nsl], ident[:, 0:64], xg_t[:, nsl],
                                     start=True, stop=False, tile_position=(0, 0))
                    nc.tensor.matmul(ps[64:128, nsl], ident[:, 64:128],
                                     xg_t[:, nsl], start=True, stop=False,
                                     tile_position=(0, 64))

            xg_tiles = {0: fetch_xg(0)}
            if T_steps > 1:
                xg_tiles[1] = fetch_xg(1)
            ps = rgpsp.tile([128, 1536], f32, tag="rg")
            id_mms(ps, xg_tiles[0])

            CORDER = [0, 4, 1, 5, 2, 6, 3, 7]
            for t in range(T_steps):
                xg_t = xg_tiles.pop(t)
                # recurrent matmuls, bank-major [r, h, z] so the r/h banks
                # complete early and the gate chain overlaps the z pass.
                # k-chunk c uses hT slot [:, c%4, (c//4)*64:+64]
                for ns in (1, 2, 0):
                    nsl = slice(ns * 512, ns * 512 + 512)
                    for ci, c in enumerate(CORDER):
                        lhsT = hT_prev[:, c % 4, (c // 4) * 64:(c // 4) * 64 + 64]
                        st = ns == 2 and ci == 0
                        sp = ci == len(CORDER) - 1
                        nc.tensor.matmul(
                            ps[0:64, nsl], lhsT, Up_sb[:, 0, c, nsl],
                            start=st, stop=sp, tile_position=(0, 0))
                        nc.tensor.matmul(
                            ps[64:128, nsl], lhsT, Up_sb[:, 1, c, nsl],
                            start=st, stop=sp, tile_position=(0, 64))

                # hoist next step's identity MMs (xg prefetched 2 steps out)
                # so the PE queue has work during this step's gate chain
                if t + 2 < T_steps:
                    xg_tiles[t + 2] = fetch_xg(t + 2)
                if t + 1 < T_steps:
                    ps_nxt = rgpsp.tile([128, 1536], f32, tag="rg")
                    with tc.high_priority(offset=300):
                        id_mms(ps_nxt, xg_tiles[t + 1])
                else:
                    ps_nxt = None

                h_new = hpool.tile([128, H2], f32, tag="h")
                hT_new = hTpool.tile([128, 4, 128], f16, tag="hT")

                for blk in range(4):
                    bsl = slice(blk * 128, blk * 128 + 128)       # j0 block
                    zsl = slice(blk * 128, blk * 128 + 128)
                    rsl = slice(512 + blk * 128, 512 + blk * 128 + 128)
                    hsl = slice(1024 + blk * 128, 1024 + blk * 128 + 128)

                    r_t = gtp.tile([128, 128], f32, tag="r")
                    zc_t = gtp.tile([128, 128], f32, tag="zc")
                    th_t = gtp.tile([128, 128], f32, tag="th")
                    hh_t = gtp.tile([128, 128], f32, tag="hh")
                    d_t = gtp.tile([128, 128], f32, tag="d")
                    e_t = gtp.tile([128, 128], f32, tag="e")

                    # chain: sigmoid(r) -> th1 -> th2 -> tanh -> d -> e -> hn
                    nc.scalar.activation(out=r_t[:], in_=ps[:, rsl], func=Sigmoid)
                    if with_bias_rh:
                        ph_t = gtp.tile([128, 128], f32, tag="ph")
                        nc.vector.tensor_add(out=ph_t[:], in0=ps[:, hsl],
                                             in1=brh_sb[:, bsl])
                        nc.vector.tensor_mul(out=th_t[:], in0=r_t[:], in1=ph_t[:])
                    else:
                        nc.vector.tensor_mul(out=th_t[:], in0=r_t[:], in1=ps[:, hsl])
                    nc.vector.tensor_add(out=th_t[:], in0=th_t[:],
                                         in1=xg_t[:, hsl])
                    # zc = 1 - z, computed on ACT while DVE does th1/th2
                    nc.scalar.activation(out=zc_t[:], in_=ps[:, zsl], func=Sigmoid,
                                         scale=-1.0)
                    nc.scalar.activation(out=hh_t[:], in_=th_t[:], func=Tanh)
                    # d = hh - h (gpsimd keeps DVE free for e/hn)
                    nc.gpsimd.tensor_tensor(out=d_t[:], in0=hh_t[:],
                                            in1=h_prev[:, bsl],
                                            op=mybir.AluOpType.subtract)
                    nc.vector.tensor_mul(out=e_t[:], in0=zc_t[:], in1=d_t[:])
                    nc.vector.tensor_add(out=h_new[:, bsl], in0=h_prev[:, bsl],
                                         in1=e_t[:])

                    # transpose this block -> hT slot
                    tp = trpsp.tile([128, 128], f32, tag="tr")
                    nc.tensor.transpose(tp[:], h_new[:, bsl], ident32[:])
                    nc.scalar.copy(out=hT_new[:, blk, :], in_=tp[:])

                # stream out
                nc.sync.dma_start(ys_e[t, :, 0:256], h_new[:, 0:256])
                nc.sync.dma_start(ys_e[t, :, 256:512], h_new[:, 256:512])

                h_prev = h_new
                hT_prev = hT_new
                ps = ps_nxt

        singles_cm.__exit__(None, None, None)

    nc.compile()
    return nc


def _get_nc(T_steps, with_bias_pre, with_bias_rh):
    key = (T_steps, with_bias_pre, with_bias_rh)
    if key not in _CACHE:
        _CACHE[key] = _build(T_steps, with_bias_pre, with_bias_rh)
    return _CACHE[key]


def _prep_inputs(x, hidden, W, U, bi, br, T_steps):
    """Host-side layout permutations. Returns per-core in_maps."""
    f = np.float32
    x = np.asarray(x, f)
    hidden = np.asarray(hidden, f)
    W = np.asarray(W, f)
    U = np.asarray(U, f)
    bi = np.asarray(bi, f)
    br = np.asarray(br, f)
    T_loc = T_steps // NCORES

    # permuted halves: half jh columns = [z | r | h] each 512 wide
    def perm_cols(M):
        # M: [*, 3H] -> [2, *, 1536]
        out = np.empty((2, M.shape[0], 1536), f)
        for jh in range(2):
            for g in range(3):
                out[jh, :, g * 512:(g + 1) * 512] = \
                    M[:, g * H + jh * 512: g * H + jh * 512 + 512]
        return out

    Uh = perm_cols(U)  # [2, 1024, 1536]
    Wh = perm_cols(W)  # [2, 256, 1536]
    # partition-major device layouts
    Up = np.ascontiguousarray(
        Uh.reshape(2, 8, 128, 1536).transpose(2, 0, 1, 3)).astype(np.float16)
    Wp = np.ascontiguousarray(
        Wh.reshape(2, 2, 128, 1536).transpose(2, 0, 1, 3)).astype(np.float16)

    # x transposed: xT[t, p, dk, b] = x[b, t, dk*128+p]
    xT = np.ascontiguousarray(
        x.transpose(1, 2, 0).reshape(T_steps, 2, 128, B).transpose(0, 2, 1, 3)
    ).astype(np.float16)

    # h0 layouts
    h0bt = np.concatenate([hidden[:, 0:H2], hidden[:, H2:H]], axis=0)  # [128,512]
    h0bt = np.ascontiguousarray(h0bt)
    # h0T[p, c, jh*64+b] = hidden[b, jh*512 + c*128 + p]
    h0T = np.empty((128, 4, 128), np.float16)
    for c in range(4):
        for jh in range(2):
            h0T[:, c, jh * 64:jh * 64 + 64] = \
                hidden[:, jh * 512 + c * 128: jh * 512 + (c + 1) * 128].T

    # biases
    bz = bi[0:H] + br[0:H]
    brr = bi[H:2 * H] + br[H:2 * H]
    bhi = bi[2 * H:3 * H]
    bhr = br[2 * H:3 * H]
    with_bias_pre = bool(np.any(bz) or np.any(brr) or np.any(bhi))
    with_bias_rh = bool(np.any(bhr))

    base = {"Up": Up, "Wp": Wp, "h0bt": h0bt, "h0T": h0T}
    if with_bias_pre:
        row = np.empty((2, 1536), f)
        for jh in range(2):
            row[jh, 0:512] = bz[jh * 512:jh * 512 + 512]
            row[jh, 512:1024] = brr[jh * 512:jh * 512 + 512]
            row[jh, 1024:1536] = bhi[jh * 512:jh * 512 + 512]
        bpre = np.empty((128, 1536), f)
        bpre[0:64] = row[0]
        bpre[64:128] = row[1]
        base["bpre"] = bpre
    if with_bias_rh:
        brh = np.empty((128, H2), f)
        brh[0:64] = bhr[0:H2]
        brh[64:128] = bhr[H2:H]
        base["brh"] = brh

    in_maps = []
    for cid in range(NCORES):
        m = dict(base)
        m["xT"] = np.ascontiguousarray(xT[cid * T_loc:(cid + 1) * T_loc])
        in_maps.append(m)
    return in_maps, with_bias_pre, with_bias_rh


def run(x, hidden, W, U, bi, br, T_steps=T, trace=False):
    from concourse.bass_utils import run_bass_kernel_spmd

    in_maps, wbp, wbr = _prep_inputs(x, hidden, W, U, bi, br, T_steps)
    nc = _get_nc(T_steps, wbp, wbr)
    res = run_bass_kernel_spmd(nc, in_maps, core_ids=list(range(NCORES)),
                               trace=trace)
    o = res.results[0]["ys"]  # [T, 128, 512]
    ys = np.concatenate([o[:, 0:64, :], o[:, 64:128, :]], axis=2)  # [T,64,1024]
    output = np.ascontiguousarray(ys.transpose(1, 0, 2))  # [B, T, H]
    state = np.ascontiguousarray(ys[T_steps - 1])  # [64, 1024]
    return (output, state), res


def kernel(x, hidden, W, U, bi, br):
    (output, state), _ = run(x, hidden, W, U, bi, br, T_steps=T, trace=False)
    return output, state


# revision 19
# speedup vs baseline: 1.2657x; 1.2657x over previous
"""GRU (Keras reset_after=True) encoder kernel for 8 trn2 NeuronCores.

Strategy:
  - x@W projection (xg) sharded across cores by time, then one AllGather.
  - The sequential recurrence is REPLICATED on all 8 cores (per-step
    collectives are too expensive: ~4.6us floor x 512 steps).
  - Per step, the recurrent matmul h @ U streams U through the PE with the
    batch (64) as the stationary operand, 2-way column-tiled so PE array
    columns 0-63 serve the H-halve jh=0 and columns 64-127 serve jh=1.
  - xz/xr are accumulated into PSUM via an identity matmul so sigmoid reads
    PSUM directly; gates are pipelined in 4 blocks of 128 columns so PE
    transposes + next-step matmuls start before all gates finish.

Layouts (host-prepared):
  gate tensors on chip: [p = b + 64*jh, j0] with j = jh*512 + j0, H=1024.
  psum rg: [128, 1536] = [z 512 | r 512 | h 512] (cols j0 per half jh(p)).
  hT_buf:  [128, 4, 128]; slot [:, c, jh*64:+64] = h^T for h-chunk 4*jh+c.
"""

import os
import sys

import numpy as np

for _p in ("/opt/trn_rl_repo",):
    if _p not in sys.path and os.path.isdir(_p):
        sys.path.insert(0, _p)

B, T, D, H = 64, 512, 256, 1024
NCORES = 8
H2 = H // 2  # 512

_CACHE = {}


def _build(T_steps, with_bias_pre, with_bias_rh, trace_scopes=False):
    import concourse.bass as bass
    import concourse.tile as tile
    from concourse import bacc, mybir
    from concourse.masks import make_identity

    f32 = mybir.dt.float32
    f16 = mybir.dt.float16
    T_loc = T_steps // NCORES

    nc = bacc.Bacc("TRN2", target_bir_lowering=False, debug=False,
                   num_devices=NCORES)

    # ---- external I/O ----
    xT_e = nc.dram_tensor("xT", [T_loc, 128, 2, 64], f16, kind="ExternalInput").ap()
    Up_e = nc.dram_tensor("Up", [128, 2, 8, 1536], f16, kind="ExternalInput").ap()
    Wp_e = nc.dram_tensor("Wp", [128, 2, 2, 1536], f16, kind="ExternalInput").ap()
    h0bt_e = nc.dram_tensor("h0bt", [128, H2], f32, kind="ExternalInput").ap()
    h0T_e = nc.dram_tensor("h0T", [128, 4, 128], f16, kind="ExternalInput").ap()
    if with_bias_pre:
        bpre_e = nc.dram_tensor("bpre", [128, 1536], f32, kind="ExternalInput").ap()
    if with_bias_rh:
        brh_e = nc.dram_tensor("brh", [128, H2], f32, kind="ExternalInput").ap()
    ys_e = nc.dram_tensor("ys", [T_steps, 128, H2], f32, kind="ExternalOutput").ap()

    with tile.TileContext(nc) as tc, \
            tc.tile_pool(name="dram", bufs=1, space="DRAM") as dramp:
        xg_shard = dramp.tile([T_loc, 128, 1536], f16, name="xg_shard")
        xg_full = dramp.tile([T_steps, 128, 1536], f16, name="xg_full",
                             addr_space="Shared")
        singles_cm = tc.tile_pool(name="singles", bufs=1)
        singles = singles_cm.__enter__()
        # constants (live for the whole kernel)
        ident = singles.tile([128, 128], f16)
        make_identity(nc, ident)
        ident32 = singles.tile([128, 128], f32)
        make_identity(nc, ident32)
        Up_sb = singles.tile([128, 2, 8, 1536], f16)
        for jh in range(2):
            for k in range(8):
                nc.sync.dma_start(Up_sb[:, jh, k], Up_e[:, jh, k])
        h0_sb = singles.tile([128, H2], f32)
        nc.sync.dma_start(h0_sb[:], h0bt_e[:])
        hT0_sb = singles.tile([128, 4, 128], f16)
        nc.sync.dma_start(hT0_sb[:], h0T_e[:])
        if with_bias_pre:
            bpre_sb = singles.tile([128, 1536], f32)
            nc.sync.dma_start(bpre_sb[:], bpre_e[:])
        if with_bias_rh:
            brh_sb = singles.tile([128, H2], f32)
            nc.sync.dma_start(brh_sb[:], brh_e[:])

        with (
            tc.tile_pool(name="presing", bufs=1) as presing,
            tc.tile_pool(name="xk", bufs=3) as xkp,
            tc.tile_pool(name="xgsb", bufs=3) as xgsbp,
            tc.tile_pool(name="prepsum", bufs=2, space="PSUM") as prepsum,
        ):
            Wp_sb = presing.tile([128, 2, 2, 1536], f16)
            for jh in range(2):
                for dk in range(2):
                    nc.sync.dma_start(Wp_sb[:, jh, dk], Wp_e[:, jh, dk])

            # ---- phase 1: xg = x @ W (+ bias folds) for local t-shard ----
            for it in range(T_loc):
                xk = xkp.tile([128, 2, 64], f16, tag="xk")
                nc.sync.dma_start(xk[:], xT_e[it])
                ps = prepsum.tile([128, 1536], f32, tag="ps")
                for dk in range(2):
                    st = dk == 0
                    sp = dk == 1
                    for ns in range(3):
                        nsl = slice(ns * 512, ns * 512 + 512)
                        nc.tensor.matmul(
                            ps[0:64, nsl], xk[:, dk, :], Wp_sb[:, 0, dk, nsl],
                            start=st, stop=sp, tile_position=(0, 0))
                        nc.tensor.matmul(
                            ps[64:128, nsl], xk[:, dk, :], Wp_sb[:, 1, dk, nsl],
                            start=st, stop=sp, tile_position=(0, 64))
                xg_sb = xgsbp.tile([128, 1536], f16, tag="xg_sb")
                for ns in range(3):
                    nsl = slice(ns * 512, ns * 512 + 512)
                    if with_bias_pre:
                        nc.vector.tensor_add(
                            out=xg_sb[:, nsl], in0=ps[:, nsl], in1=bpre_sb[:, nsl])
                    else:
                        nc.vector.tensor_copy(out=xg_sb[:, nsl], in_=ps[:, nsl])
                nc.sync.dma_start(xg_shard[it, :, 0:768], xg_sb[:, 0:768])
                nc.sync.dma_start(xg_shard[it, :, 768:1536], xg_sb[:, 768:1536])

            # ---- phase 2: allgather xg ----
            nc.gpsimd.collective_compute(
                "AllGather",
                mybir.AluOpType.bypass,
                ins=[xg_shard[:].opt()],
                outs=[xg_full[:].opt()],
                replica_groups=[list(range(NCORES))],
            )

        # ---- phase 3: recurrence ----
        with (
            tc.tile_pool(name="xgt", bufs=4) as xgtp,
            tc.tile_pool(name="hpool", bufs=3) as hpool,
            tc.tile_pool(name="hTpool", bufs=2) as hTpool,
            tc.tile_pool(name="gt", bufs=8) as gtp,
            tc.tile_pool(name="rgps", bufs=2, space="PSUM") as rgpsp,
            tc.tile_pool(name="trps", bufs=2, space="PSUM") as trpsp,
        ):
            h_prev = h0_sb
            hT_prev = hT0_sb
            Sigmoid = mybir.ActivationFunctionType.Sigmoid
            Tanh = mybir.ActivationFunctionType.Tanh

            def fetch_xg(t):
                xg_t = xgtp.tile([128, 1536], f16, tag="xg_t")
                for q in range(4):
                    csl = slice(q * 384, q * 384 + 384)
                    nc.sync.dma_start(xg_t[:, csl], xg_full[t, :, csl])
                return xg_t

            def id_mms(ps3, xg_t):
                # identity matmuls fold xz/xr into psum banks z and r
                for ns in range(2):
                    nsl = slice(ns * 512, ns * 512 + 512)
                    pst = ps3[ns]
                    nc.tensor.matmul(pst[0:64, :], ident[:, 0:64], xg_t[:, nsl],
                                     start=True, stop=False, tile_position=(0, 0))
                    nc.tensor.matmul(pst[64:128, :], ident[:, 64:128],
                                     xg_t[:, nsl], start=True, stop=False,
                                     tile_position=(0, 64))

            def alloc_ps3():
                return (rgpsp.tile([128, 512], f32, tag="rgz", name="psz"),
                        rgpsp.tile([128, 512], f32, tag="rgr", name="psr"),
                        rgpsp.tile([128, 512], f32, tag="rgh", name="psh"))

            xg_tiles = {0: fetch_xg(0)}
            if T_steps > 1:
                xg_tiles[1] = fetch_xg(1)
            ps = alloc_ps3()
            id_mms(ps, xg_tiles[0])

            CORDER = [0, 4, 1, 5, 2, 6, 3, 7]
            for t in range(T_steps):
                xg_t = xg_tiles.pop(t)
                # recurrent matmuls, bank-major [r, h, z] so the r/h banks
                # complete early and the gate chain overlaps the z pass.
                # k-chunk c uses hT slot [:, c%4, (c//4)*64:+64]
                for ns in (1, 2, 0):
                    nsl = slice(ns * 512, ns * 512 + 512)
                    pst = ps[ns]
                    for ci, c in enumerate(CORDER):
                        lhsT = hT_prev[:, c % 4, (c // 4) * 64:(c // 4) * 64 + 64]
                        st = ns == 2 and ci == 0
                        sp = ci == len(CORDER) - 1
                        nc.tensor.matmul(
                            pst[0:64, :], lhsT, Up_sb[:, 0, c, nsl],
                            start=st, stop=sp, tile_position=(0, 0))
                        nc.tensor.matmul(
                            pst[64:128, :], lhsT, Up_sb[:, 1, c, nsl],
                            start=st, stop=sp, tile_position=(0, 64))

                # hoist next step's identity MMs (xg prefetched 2 steps out)
                # so the PE queue has work during this step's gate chain
                if t + 2 < T_steps:
                    xg_tiles[t + 2] = fetch_xg(t + 2)
                if t + 1 < T_steps:
                    ps_nxt = alloc_ps3()
                    with tc.high_priority(offset=300):
                        id_mms(ps_nxt, xg_tiles[t + 1])
                else:
                    ps_nxt = None

                h_new = hpool.tile([128, H2], f32, tag="h")
                hT_new = hTpool.tile([128, 4, 128], f16, tag="hT")

                for blk in range(4):
                    bsl = slice(blk * 128, blk * 128 + 128)       # j0 block
                    psz, psr, psh = ps

                    r_t = gtp.tile([128, 128], f32, tag="r")
                    zc_t = gtp.tile([128, 128], f32, tag="zc")
                    th_t = gtp.tile([128, 128], f32, tag="th")
                    hh_t = gtp.tile([128, 128], f32, tag="hh")
                    d_t = gtp.tile([128, 128], f32, tag="d")
                    e_t = gtp.tile([128, 128], f32, tag="e")

                    # chain: sigmoid(r) -> th1 -> th2 -> tanh -> d -> e -> hn
                    nc.scalar.activation(out=r_t[:], in_=psr[:, bsl], func=Sigmoid)
                    if with_bias_rh:
                        ph_t = gtp.tile([128, 128], f32, tag="ph")
                        nc.vector.tensor_add(out=ph_t[:], in0=psh[:, bsl],
                                             in1=brh_sb[:, bsl])
                        nc.vector.tensor_mul(out=th_t[:], in0=r_t[:], in1=ph_t[:])
                    else:
                        nc.vector.tensor_mul(out=th_t[:], in0=r_t[:], in1=psh[:, bsl])
                    nc.vector.tensor_add(out=th_t[:], in0=th_t[:],
                                         in1=xg_t[:, 1024 + blk * 128:
                                                   1024 + blk * 128 + 128])
                    # zc = 1 - z, computed on ACT while DVE does th1/th2
                    nc.scalar.activation(out=zc_t[:], in_=psz[:, bsl],
                                         func=Sigmoid, scale=-1.0)
                    nc.scalar.activation(out=hh_t[:], in_=th_t[:], func=Tanh)
                    # d = hh - h (gpsimd keeps DVE free for e/hn)
                    nc.gpsimd.tensor_tensor(out=d_t[:], in0=hh_t[:],
                                            in1=h_prev[:, bsl],
                                            op=mybir.AluOpType.subtract)
                    nc.vector.tensor_mul(out=e_t[:], in0=zc_t[:], in1=d_t[:])
                    nc.vector.tensor_add(out=h_new[:, bsl], in0=h_prev[:, bsl],
                                         in1=e_t[:])

                    # transpose this block -> hT slot
                    tp = trpsp.tile([128, 128], f32, tag="tr")
                    nc.tensor.transpose(tp[:], h_new[:, bsl], ident32[:])
                    nc.scalar.copy(out=hT_new[:, blk, :], in_=tp[:])

                # stream out
                nc.sync.dma_start(ys_e[t, :, 0:256], h_new[:, 0:256])
                nc.sync.dma_start(ys_e[t, :, 256:512], h_new[:, 256:512])

                h_prev = h_new
                hT_prev = hT_new
                ps = ps_nxt

        singles_cm.__exit__(None, None, None)

    nc.compile()
    return nc


def _get_nc(T_steps, with_bias_pre, with_bias_rh):
    key = (T_steps, with_bias_pre, with_bias_rh)
    if key not in _CACHE:
        _CACHE[key] = _build(T_steps, with_bias_pre, with_bias_rh)
    return _CACHE[key]


def _prep_inputs(x, hidden, W, U, bi, br, T_steps):
    """Host-side layout permutations. Returns per-core in_maps."""
    f = np.float32
    x = np.asarray(x, f)
    hidden = np.asarray(hidden, f)
    W = np.asarray(W, f)
    U = np.asarray(U, f)
    bi = np.asarray(bi, f)
    br = np.asarray(br, f)
    T_loc = T_steps // NCORES

    # permuted halves: half jh columns = [z | r | h] each 512 wide
    def perm_cols(M):
        # M: [*, 3H] -> [2, *, 1536]
        out = np.empty((2, M.shape[0], 1536), f)
        for jh in range(2):
            for g in range(3):
                out[jh, :, g * 512:(g + 1) * 512] = \
                    M[:, g * H + jh * 512: g * H + jh * 512 + 512]
        return out

    Uh = perm_cols(U)  # [2, 1024, 1536]
    Wh = perm_cols(W)  # [2, 256, 1536]
    # partition-major device layouts
    Up = np.ascontiguousarray(
        Uh.reshape(2, 8, 128, 1536).transpose(2, 0, 1, 3)).astype(np.float16)
    Wp = np.ascontiguousarray(
        Wh.reshape(2, 2, 128, 1536).transpose(2, 0, 1, 3)).astype(np.float16)

    # x transposed: xT[t, p, dk, b] = x[b, t, dk*128+p]
    xT = np.ascontiguousarray(
        x.transpose(1, 2, 0).reshape(T_steps, 2, 128, B).transpose(0, 2, 1, 3)
    ).astype(np.float16)

    # h0 layouts
    h0bt = np.concatenate([hidden[:, 0:H2], hidden[:, H2:H]], axis=0)  # [128,512]
    h0bt = np.ascontiguousarray(h0bt)
    # h0T[p, c, jh*64+b] = hidden[b, jh*512 + c*128 + p]
    h0T = np.empty((128, 4, 128), np.float16)
    for c in range(4):
        for jh in range(2):
            h0T[:, c, jh * 64:jh * 64 + 64] = \
                hidden[:, jh * 512 + c * 128: jh * 512 + (c + 1) * 128].T

    # biases
    bz = bi[0:H] + br[0:H]
    brr = bi[H:2 * H] + br[H:2 * H]
    bhi = bi[2 * H:3 * H]
    bhr = br[2 * H:3 * H]
    with_bias_pre = bool(np.any(bz) or np.any(brr) or np.any(bhi))
    with_bias_rh = bool(np.any(bhr))

    base = {"Up": Up, "Wp": Wp, "h0bt": h0bt, "h0T": h0T}
    if with_bias_pre:
        row = np.empty((2, 1536), f)
        for jh in range(2):
            row[jh, 0:512] = bz[jh * 512:jh * 512 + 512]
            row[jh, 512:1024] = brr[jh * 512:jh * 512 + 512]
            row[jh, 1024:1536] = bhi[jh * 512:jh * 512 + 512]
        bpre = np.empty((128, 1536), f)
        bpre[0:64] = row[0]
        bpre[64:128] = row[1]
        base["bpre"] = bpre
    if with_bias_rh:
        brh = np.empty((128, H2), f)
        brh[0:64] = bhr[0:H2]
        brh[64:128] = bhr[H2:H]
        base["brh"] = brh

    in_maps = []
    for cid in range(NCORES):
        m = dict(base)
        m["xT"] = np.ascontiguousarray(xT[cid * T_loc:(cid + 1) * T_loc])
        in_maps.append(m)
    return in_maps, with_bias_pre, with_bias_rh


def run(x, hidden, W, U, bi, br, T_steps=T, trace=False):
    from concourse.bass_utils import run_bass_kernel_spmd

    in_maps, wbp, wbr = _prep_inputs(x, hidden, W, U, bi, br, T_steps)
    nc = _get_nc(T_steps, wbp, wbr)
    res = run_bass_kernel_spmd(nc, in_maps, core_ids=list(range(NCORES)),
                               trace=trace)
    o = res.results[0]["ys"]  # [T, 128, 512]
    ys = np.concatenate([o[:, 0:64, :], o[:, 64:128, :]], axis=2)  # [T,64,1024]
    output = np.ascontiguousarray(ys.transpose(1, 0, 2))  # [B, T, H]
    state = np.ascontiguousarray(ys[T_steps - 1])  # [64, 1024]
    return (output, state), res


def kernel(x, hidden, W, U, bi, br):
    (output, state), _ = run(x, hidden, W, U, bi, br, T_steps=T, trace=False)
    return output, state
